# revision 24
# baseline (speedup 1.0000x reference)
"""AttentionBlock (GroupNorm + 8-head attention + proj + residual) on 8 TRN2 cores.

Sharding: data-parallel over batch B=8 -> one image per NeuronCore, weights
replicated, no collectives.

Perf design (vs bf16 baseline):
- All GEMMs in fp8e4m3 with DoubleRow perf mode (2 k-tiles per matmul at
  0.5 cyc/row; 64-partition DR runs 2 rows/cycle on HW).
- The K=64 S=K^T Q matmul uses the 2nd DR k-tile as a bias injector
  (lhsT tile = BS/64 const, rhs tile = ones) so PE emits S' = A*S + BS
  directly, where A = 8/ln2 (Schraudolph exponent scale, folded into the
  host-side q/k weight scales).
- Softmax exp is split across three engines: ACT computes true exp
  (scale/bias in the activation op), DVE/Pool compute Schraudolph exp:
  clamp(S',0,119) -> uint8 -> bitcast fp8e4m3 == exp(S)*const. The per-path
  constant offset is calibrated (KAPPA) so paths can mix within a head;
  softmax normalization cancels residual per-head factors.
- fp8 weights pre-scaled by powers of 2 into e4m3's normal range (denormal
  flush otherwise dominates error); compensated in the psum->fp8 casts.
"""
import sys
import types

import numpy as np
import ml_dtypes

import concourse.bass as bass
import concourse.tile as tile
from concourse import bacc, mybir
from concourse.bass_utils import run_bass_kernel_spmd

F32 = mybir.dt.float32
BF16 = mybir.dt.bfloat16
F8 = mybir.dt.float8e4
U8 = mybir.dt.uint8

B, C, N = 8, 512, 1024          # batch, channels, H*W
NH, HD = 8, 64                  # heads, head_dim
G, GS = 32, 16                  # groups, channels per group
EPS = 1e-5
NCORES = 8
CT = C // 128                   # 4 channel tiles
ST = N // 128                   # 8 s-tiles
TRACE = False
DEBUG = False

A_SCH = 8.0 / np.log(2.0)       # Schraudolph exponent scale
SQA = np.sqrt(A_SCH)
BS = 40.0                       # Schraudolph offset (64 * 0.625)
KAPPA = 0.2600971698471845      # Schraudolph path multiplicative offset
ACT_BIAS = float(-BS / A_SCH + np.log(KAPPA))   # -4.81244
ACT_SCALE = float(1.0 / A_SCH)
QK_COMP = 1.0 / 16.0            # q/k weights pre-scaled x16
PROJ_COMP = 1.0 / 256.0         # pw x4, h x64

# P-tile engine map per (head, st): 'a' ACT exp, 'd' DVE, 'p' Pool
PMAP = [
    ['a', 'd', 'a', 'd', 'a', 'd', 'a', 'd'],
    ['d', 'a', 'd', 'a', 'd', 'a', 'd', 'a'],
] * 4

_CACHE = {}


def _install_ntff_hook():
    if "antenv.axon_hooks" in sys.modules:
        return
    try:
        from trn_agent_boot.trn_boot import _ntff_profile_via_ctypes
        hook = _ntff_profile_via_ctypes("/opt/axon/libaxon_pjrt.so")
    except Exception:
        hook = None
    mod = types.ModuleType("antenv.axon_hooks")
    mod.get_axon_ntff_profile_hook = lambda: hook
    mod.set_axon_ntff_profile_hook = lambda h: None
    sys.modules["antenv.axon_hooks"] = mod


def build_nc(debug=False):
    nc = bacc.Bacc("TRN2", target_bir_lowering=False, debug=False,
                   num_devices=NCORES)
    x = nc.dram_tensor("x", (C, N), BF16, kind="ExternalInput").ap()
    xpb = nc.dram_tensor("xpb", (C, N), F32, kind="ExternalInput").ap()
    qkvw = nc.dram_tensor("qkvw", (C, 3 * C), F8, kind="ExternalInput").ap()
    pw = nc.dram_tensor("pw", (C, C), F8, kind="ExternalInput").ap()
    gnw = nc.dram_tensor("gnw", (128, CT), F32, kind="ExternalInput").ap()
    gnb = nc.dram_tensor("gnb", (128, CT), F32, kind="ExternalInput").ap()
    mask = nc.dram_tensor("mask", (128, 128), F32, kind="ExternalInput").ap()
    out = nc.dram_tensor("out", (C, N), F32, kind="ExternalOutput").ap()
    rs_scr = nc.dram_tensor("rs_scr", (NH, N), F32).ap()   # raw rowsums
    rs_scr2 = nc.dram_tensor("rs_scr2", (NH, N), F32).ap()  # reciprocals

    dbg = {}
    if debug:
        for name, shape in [("d_xn", (C, N)), ("d_q", (C, N)), ("d_k", (C, N)),
                            ("d_vt", (N, NH * 65)), ("d_h", (C, N)),
                            ("d_p", (128, NH * ST * 8))]:
            dbg[name] = nc.dram_tensor(name, shape, F32, kind="ExternalOutput").ap()

    x_t = x.rearrange("(t p) n -> p t n", p=128)
    xpb_t = xpb.rearrange("(t p) n -> p t n", p=128)
    qkvw_t = qkvw.rearrange("(t p) o -> p t o", p=128)
    pw_t = pw.rearrange("(t p) o -> p t o", p=128)
    out_t = out.rearrange("(t p) n -> p t n", p=128)

    DR = mybir.MatmulPerfMode.DoubleRow

    with tile.TileContext(nc) as tc:
        with (
            tc.tile_pool(name="wpool", bufs=1) as wp,       # persistent
            tc.tile_pool(name="small", bufs=1) as sm,       # consts/stats
            tc.tile_pool(name="rsbp", bufs=2) as rsbp,      # rs broadcast tiles
            tc.tile_pool(name="rsep", bufs=5) as rsep,      # hraw sbuf copies
            tc.tile_pool(name="rspp", bufs=2) as rspp,      # rs pair tiles
            tc.tile_pool(name="outp", bufs=2) as op_,       # output tiles
            tc.tile_pool(name="dbgp", bufs=2) as dbgp,      # debug dumps
            tc.tile_pool(name="ps_x", bufs=1, space="PSUM") as ps_x,   # 1 bank
            tc.tile_pool(name="ps_h", bufs=1, space="PSUM") as ps_h,   # 1 bank
            tc.tile_pool(name="ps_s", bufs=3, space="PSUM") as ps_s,   # 6 banks
        ):
            # ---- persistent SBUF ----
            x_sb = wp.tile([128, CT, N], BF16, tag="xbf")
            xpb_sb = wp.tile([128, CT, N], F32, tag="xpb")
            xn8 = wp.tile([128, CT, N], F8, tag="xn8")
            qkvw_sb = wp.tile([128, CT, 3 * C], F8, tag="qkvw")
            pw_sb = wp.tile([128, CT, C], F8, tag="pw")
            q8 = wp.tile([128, 5, N], F8, tag="q8")     # slot 4 = ones
            k8 = wp.tile([128, 5, N], F8, tag="k8")     # slot 4 = BS/64
            vt8 = wp.tile([128, ST, NH, 66], F8, tag="vt8")  # 66: DR ldweights stride must be 16B-aligned
            p8 = wp.tile([128, NH, ST, N], F8, tag="p8")
            h8 = wp.tile([128, CT, N], F8, tag="h8")
            gnw_sb = wp.tile([128, CT], F32, tag="gnw")
            gnb_sb = wp.tile([128, CT], F32, tag="gnb")
            mask_sb = wp.tile([128, 128], F32, tag="mask")

            # ---- input DMAs ----
            for ct in range(CT):
                nc.sync.dma_start(out=x_sb[:, ct, :], in_=x_t[:, ct, :])
            nc.sync.dma_start(out=qkvw_sb, in_=qkvw_t)
            nc.sync.dma_start(out=pw_sb, in_=pw_t)
            nc.sync.dma_start(out=gnw_sb, in_=gnw)
            nc.sync.dma_start(out=gnb_sb, in_=gnb)
            nc.sync.dma_start(out=mask_sb, in_=mask)
            for ct in range(CT):
                nc.sync.dma_start(out=xpb_sb[:, ct, :], in_=xpb_t[:, ct, :])

            # ---- GroupNorm stats (first on DVE queue: critical path) ----
            stats_in = sm.tile([128, 8], F32, tag="sin")
            eps_t = sm.tile([128, 1], F32, tag="eps")
            bias_t = sm.tile([128, 1], F32, tag="biast")
            for ct in range(CT):
                stats = sm.tile([128, 2, 6], F32, tag="bst")
                for j in range(2):
                    nc.vector.bn_stats(out=stats[:, j, :],
                                       in_=x_sb[:, ct, j * 512:(j + 1) * 512])
                mv = sm.tile([128, 2], F32, tag="mv")
                nc.vector.bn_aggr(out=mv, in_=stats)
                nc.vector.tensor_copy(stats_in[:, ct:ct + 1], mv[:, 0:1])
                msq = sm.tile([128, 1], F32, tag="msq")
                nc.vector.tensor_mul(msq, mv[:, 0:1], mv[:, 0:1])
                nc.vector.tensor_add(stats_in[:, 4 + ct:5 + ct], mv[:, 1:2], msq)
            nc.vector.memset(eps_t, EPS)
            nc.vector.memset(bias_t, ACT_BIAS)
            nc.gpsimd.memset(q8[:, 4, :], 1.0)
            nc.gpsimd.memset(k8[:, 4, :], 0.625)
            nc.gpsimd.memset(vt8[:, :, :, 64:65], 0.25)
            stats_ps = ps_x.tile([128, 8], F32, tag="px")
            nc.tensor.matmul(stats_ps, mask_sb, stats_in, start=True, stop=True)
            stats_gs = sm.tile([128, 8], F32, tag="sgs")
            nc.vector.tensor_copy(stats_gs, stats_ps)
            means_g = stats_gs[:, 0:4]
            e2_g = stats_gs[:, 4:8]
            msq_g = sm.tile([128, 4], F32, tag="msqg")
            nc.vector.tensor_mul(msq_g, means_g, means_g)
            var_g = sm.tile([128, 4], F32, tag="varg")
            nc.vector.tensor_tensor(out=var_g, in0=e2_g, in1=msq_g,
                                    op=mybir.AluOpType.subtract)
            lnv = sm.tile([128, 4], F32, tag="lnv")
            nc.scalar.activation(out=lnv, in_=var_g,
                                 func=mybir.ActivationFunctionType.Ln,
                                 bias=eps_t, scale=1.0)
            rstd = sm.tile([128, 4], F32, tag="rstd")
            nc.scalar.activation(out=rstd, in_=lnv,
                                 func=mybir.ActivationFunctionType.Exp,
                                 bias=0.0, scale=-0.5)
            sc_g = sm.tile([128, 4], F32, tag="scg")
            nc.vector.tensor_mul(sc_g, rstd, gnw_sb)
            tmp_b = sm.tile([128, 4], F32, tag="tmpb")
            nc.vector.tensor_mul(tmp_b, means_g, sc_g)
            bias_g = sm.tile([128, 4], F32, tag="biag")
            nc.vector.tensor_tensor(out=bias_g, in0=gnb_sb, in1=tmp_b,
                                    op=mybir.AluOpType.subtract)
            # xn8 = fp8(x * sc + bias), split Pool/DVE (both SBUF->SBUF)
            for ct in range(CT):
                e = nc.gpsimd if ct % 2 == 0 else nc.vector
                e.tensor_scalar(
                    out=xn8[:, ct, :], in0=x_sb[:, ct, :],
                    scalar1=sc_g[:, ct:ct + 1], scalar2=bias_g[:, ct:ct + 1],
                    op0=mybir.AluOpType.mult, op1=mybir.AluOpType.add)

            # ---- stage emitters ----
            def qk_tile(pair, dst_i, ch, pool):
                dst8, base = ((q8, 0), (k8, C))[dst_i]
                pt = pool.tile([128, 512], F32,
                               tag="s" if pool is ps_s else "px")
                for i in range(2):
                    nc.tensor.matmul(
                        pt,
                        qkvw_sb[:, 2 * i:2 * i + 2,
                                base + pair * 128:base + (pair + 1) * 128],
                        xn8[:, 2 * i:2 * i + 2, ch * 512:(ch + 1) * 512],
                        start=(i == 0), stop=(i == 1), perf_mode=DR)
                if dst_i == 0:
                    nc.scalar.activation(
                        out=dst8[:, pair, ch * 512:(ch + 1) * 512], in_=pt,
                        func=mybir.ActivationFunctionType.Copy,
                        bias=0.0, scale=QK_COMP)
                else:
                    nc.vector.tensor_scalar(
                        out=dst8[:, pair, ch * 512:(ch + 1) * 512],
                        in0=pt, scalar1=QK_COMP, scalar2=None,
                        op0=mybir.AluOpType.mult)

            def vt_tile(st):
                pt = ps_h.tile([128, 512], F32, tag="ph")
                for i in range(2):
                    nc.tensor.matmul(
                        pt,
                        xn8[:, 2 * i:2 * i + 2, st * 128:(st + 1) * 128],
                        qkvw_sb[:, 2 * i:2 * i + 2, 2 * C:3 * C],
                        start=(i == 0), stop=(i == 1), perf_mode=DR)
                if st % 2 == 0:
                    nc.scalar.activation(
                        out=vt8[:, st, :, 0:64],
                        in_=pt.rearrange("p (h c) -> p h c", h=NH),
                        func=mybir.ActivationFunctionType.Copy,
                        bias=0.0, scale=1.0)
                else:
                    nc.vector.tensor_copy(
                        vt8[:, st, :, 0:64],
                        pt.rearrange("p (h c) -> p h c", h=NH))

            def s_tile(pair, st, h01):
                head = 2 * pair + h01
                lo = 64 * h01
                spt = ps_s.tile([128, N], F32, tag="s")
                for ch in range(2):
                    nc.tensor.matmul(
                        spt[:, ch * 512:(ch + 1) * 512],
                        k8[lo:lo + 64, pair:5:(4 - pair),
                           st * 128:(st + 1) * 128],
                        q8[lo:lo + 64, pair:5:(4 - pair),
                           ch * 512:(ch + 1) * 512],
                        start=True, stop=True, perf_mode=DR)
                eng = PMAP[head][st]
                pslice = p8[:, head, st, :]
                if eng == 'a':
                    nc.scalar.activation(
                        out=pslice, in_=spt,
                        func=mybir.ActivationFunctionType.Exp,
                        bias=bias_t, scale=ACT_SCALE)
                else:
                    e = nc.vector if eng == 'd' else nc.gpsimd
                    e.tensor_scalar(
                        out=pslice.bitcast(U8), in0=spt,
                        scalar1=0.0, scalar2=119.0,
                        op0=mybir.AluOpType.max,
                        op1=mybir.AluOpType.min)

            hcps = {}    # (head, ch) -> SBUF bf16 copy of H psum

            def h_chunk(pair, h01, ch, pool):
                head = 2 * pair + h01
                hpt = pool.tile([65, 512], F32,
                                tag="s" if pool is ps_s else "ph")
                for j in range(4):
                    nc.tensor.matmul(
                        hpt,
                        vt8[:, 2 * j:2 * j + 2, head, 0:65],
                        p8[:, head, 2 * j:2 * j + 2, ch * 512:(ch + 1) * 512],
                        start=(j == 0), stop=(j == 3), perf_mode=DR)
                hcp = rsep.tile([65, 512], F32, tag="hcp")
                if h01 == 0:
                    nc.scalar.copy(out=hcp, in_=hpt)
                else:
                    nc.vector.tensor_copy(hcp, hpt)
                hcps[(head, ch)] = hcp
                nc.sync.dma_start(
                    out=rs_scr[head:head + 1, ch * 512:(ch + 1) * 512],
                    in_=hcp[64:65, :])

            def rs_chain(pair):
                rs_pair = rspp.tile([2, N], F32, tag="rsp")
                nc.sync.dma_start(out=rs_pair,
                                  in_=rs_scr[2 * pair:2 * pair + 2, :])
                rs_ipair = rspp.tile([2, N], F32, tag="rsi")
                nc.vector.reciprocal_approx_fast(out=rs_ipair, in_=rs_pair)
                nc.sync.dma_start(out=rs_scr2[2 * pair:2 * pair + 2, :],
                                  in_=rs_ipair)

            def norm_chunks(pair, tail=False):
                for h01 in range(2):
                    head = 2 * pair + h01
                    rsb = rsbp.tile([64, N], F32, tag="rsb")
                    nc.sync.dma_start(
                        out=rsb,
                        in_=rs_scr2[head:head + 1, :].to_broadcast([64, N]))
                    for ch in range(2):
                        e = nc.vector if (tail and ch == 1) else nc.gpsimd
                        e.tensor_tensor(
                            out=h8[h01 * 64:(h01 + 1) * 64, pair,
                                   ch * 512:(ch + 1) * 512],
                            in0=hcps[(head, ch)][0:64, :],
                            in1=rsb[:, ch * 512:(ch + 1) * 512],
                            op=mybir.AluOpType.mult)

            osb_acc = wp.tile([128, CT, N], F32, tag="osbacc")

            def proj_tile(ot, ch, phase, pool):
                pt = pool.tile([128, 512], F32,
                               tag="s" if pool is ps_s else "px")
                i = phase
                nc.tensor.matmul(
                    pt,
                    pw_sb[:, 2 * i:2 * i + 2, ot * 128:(ot + 1) * 128],
                    h8[:, 2 * i:2 * i + 2, ch * 512:(ch + 1) * 512],
                    start=True, stop=True, perf_mode=DR)
                if phase == 0:
                    nc.vector.scalar_tensor_tensor(
                        out=osb_acc[:, ot, ch * 512:(ch + 1) * 512],
                        in0=pt, scalar=PROJ_COMP,
                        in1=xpb_sb[:, ot, ch * 512:(ch + 1) * 512],
                        op0=mybir.AluOpType.mult, op1=mybir.AluOpType.add)
                else:
                    osb = op_.tile([128, 512], F32, tag="osb")
                    if ch == 0:
                        nc.vector.scalar_tensor_tensor(
                            out=osb, in0=pt, scalar=PROJ_COMP,
                            in1=osb_acc[:, ot, ch * 512:(ch + 1) * 512],
                            op0=mybir.AluOpType.mult, op1=mybir.AluOpType.add)
                    else:
                        ptmp = op_.tile([128, 512], F32, tag="ptmp")
                        nc.scalar.activation(
                            out=ptmp, in_=pt,
                            func=mybir.ActivationFunctionType.Copy,
                            bias=0.0, scale=PROJ_COMP)
                        nc.gpsimd.tensor_tensor(
                            out=osb, in0=ptmp,
                            in1=osb_acc[:, ot, ch * 512:(ch + 1) * 512],
                            op=mybir.AluOpType.add)
                    nc.sync.dma_start(
                        out=out_t[:, ot, ch * 512:(ch + 1) * 512], in_=osb)

            # ---- pipeline ----
            ones1 = sm.tile([1, 64], BF16, tag="ones1")
            nc.vector.memset(ones1, 1.0)

            # qk(0) through the deep ps_s pool (prologue, PE idle anyway)
            for dst_i in range(2):
                for ch in range(2):
                    qk_tile(0, dst_i, ch, ps_s)

            for pair in range(4):
                for st in range(ST):
                    for h01 in range(2):
                        s_tile(pair, st, h01)
                    if pair == 0:
                        vt_tile(st)
                        if st % 2 == 1:
                            qk_tile(1, st // 4, (st // 2) % 2, ps_x)
                    else:
                        if 1 <= st <= 4:
                            i = st - 1
                            h_chunk(pair - 1, i // 2, i % 2, ps_h)
                        if st == 5:
                            rs_chain(pair - 1)
                        if pair < 3 and st in (3, 5, 6, 7):
                            i = (3, 5, 6, 7).index(st)
                            qk_tile(pair + 1, i // 2, i % 2, ps_x)
                        if pair == 3:
                            proj_tile(st // 2, st % 2, 0, ps_x)
                        if st == 7:
                            norm_chunks(pair - 1)

            # ---- tail: H(3), engine-only rs chain, proj phase 1 ----
            for h01 in range(2):
                for ch in range(2):
                    h_chunk(3, h01, ch, ps_s)
            for h01 in range(2):
                head = 6 + h01
                for ch in range(2):
                    rse = rspp.tile([1, 512], F32, tag="rse")
                    nc.vector.tensor_copy(rse, hcps[(head, ch)][64:65, :])
                    rsi = rspp.tile([1, 512], F32, tag="rsi")
                    nc.vector.reciprocal_approx_fast(out=rsi, in_=rse)
                    rsib = rspp.tile([1, 512], BF16, tag="rsib")
                    nc.vector.tensor_copy(rsib, rsi)
                    bpool = ps_x if ch == 0 else ps_h
                    rsb_ps = bpool.tile([64, 512], F32,
                                        tag="px" if ch == 0 else "ph")
                    nc.tensor.matmul(rsb_ps, ones1, rsib,
                                     start=True, stop=True)
                    nc.vector.tensor_tensor(
                        out=h8[h01 * 64:(h01 + 1) * 64, 3,
                               ch * 512:(ch + 1) * 512],
                        in0=hcps[(head, ch)][0:64, :], in1=rsb_ps,
                        op=mybir.AluOpType.mult)
            for ot in range(CT):
                for ch in range(2):
                    proj_tile(ot, ch, 1, ps_s)

            if debug:
                for name, src in [("d_xn", xn8), ("d_h", h8)]:
                    for ct in range(CT):
                        f = dbgp.tile([128, N], F32, tag="dbgf")
                        nc.vector.tensor_copy(f, src[:, ct, :])
                        nc.sync.dma_start(out=dbg[name].rearrange(
                            "(t p) n -> p t n", p=128)[:, ct, :], in_=f)
                for name, src in [("d_q", q8), ("d_k", k8)]:
                    for ct in range(CT):
                        f = dbgp.tile([128, N], F32, tag="dbgf")
                        nc.vector.tensor_copy(f, src[:, ct, :])
                        nc.sync.dma_start(out=dbg[name].rearrange(
                            "(t p) n -> p t n", p=128)[:, ct, :], in_=f)
                for st in range(ST):
                    f = dbgp.tile([128, NH * 65], F32, tag="dbgv")
                    nc.vector.tensor_copy(
                        f.rearrange("p (h c) -> p h c", h=NH), vt8[:, st, :, 0:65])
                    nc.sync.dma_start(out=dbg["d_vt"].rearrange(
                        "(t p) c -> p t c", p=128)[:, st, :], in_=f)
                for hd_ in range(NH):
                    f = dbgp.tile([128, ST * 8], F32, tag="dbgP")
                    nc.vector.tensor_copy(
                        f.rearrange("p (s c) -> p s c", s=ST),
                        p8[:, hd_, :, 0:8])
                    nc.sync.dma_start(
                        out=dbg["d_p"][:, hd_ * ST * 8:(hd_ + 1) * ST * 8], in_=f)

            # ---- proj + residual ----
            for ot in range(CT):
                osb = op_.tile([128, N], F32, tag="osb")
                for ch in range(2):
                    pt = ps_x.tile([128, 512], F32, tag="px")
                    for i in range(2):
                        nc.tensor.matmul(
                            pt,
                            pw_sb[:, 2 * i:2 * i + 2, ot * 128:(ot + 1) * 128],
                            h8[:, 2 * i:2 * i + 2, ch * 512:(ch + 1) * 512],
                            start=(i == 0), stop=(i == 1), perf_mode=DR)
                    nc.vector.scalar_tensor_tensor(
                        out=osb[:, ch * 512:(ch + 1) * 512],
                        in0=pt, scalar=PROJ_COMP,
                        in1=xpb_sb[:, ot, ch * 512:(ch + 1) * 512],
                        op0=mybir.AluOpType.mult,
                        op1=mybir.AluOpType.add)
                nc.sync.dma_start(out=out_t[:, ot, :], in_=osb)

    nc.finalize()
    return nc


def make_in_maps(x, gn_w, gn_b, qkv_w, proj_w, proj_b):
    x = np.asarray(x, dtype=np.float32).reshape(B, C, N)
    gn_w = np.asarray(gn_w, dtype=np.float32)
    gn_b = np.asarray(gn_b, dtype=np.float32)
    qkv_w = np.asarray(qkv_w, dtype=np.float32)
    proj_w = np.asarray(proj_w, dtype=np.float32)
    proj_b = np.asarray(proj_b, dtype=np.float32)

    scale = 1.0 / np.sqrt(np.sqrt(HD))
    rows = qkv_w.reshape(NH, 3, HD, C)
    qw = rows[:, 0].reshape(C, C) * (scale * SQA * 16.0)
    kw = rows[:, 1].reshape(C, C) * (scale * SQA * 16.0)
    vw = rows[:, 2].reshape(C, C) * 16.0
    qkvw_t = np.ascontiguousarray(
        np.concatenate([qw, kw, vw], axis=0).T).astype(ml_dtypes.float8_e4m3)
    pw_t = np.ascontiguousarray((proj_w * 4.0).T).astype(ml_dtypes.float8_e4m3)
    gnw_dev = np.ascontiguousarray(gn_w.reshape(CT, 128).T)
    gnb_dev = np.ascontiguousarray(gn_b.reshape(CT, 128).T)
    mask = np.zeros((128, 128), dtype=np.float32)
    for g in range(8):
        mask[g * GS:(g + 1) * GS, g * GS:(g + 1) * GS] = 1.0 / GS

    in_maps = []
    for b in range(B):
        xc = np.ascontiguousarray(x[b])
        in_maps.append({
            "x": xc.astype(ml_dtypes.bfloat16),
            "xpb": np.ascontiguousarray(xc + proj_b[:, None]),
            "qkvw": qkvw_t, "pw": pw_t,
            "gnw": gnw_dev, "gnb": gnb_dev, "mask": mask,
        })
    return in_maps


def kernel(x, gn_w, gn_b, qkv_w, proj_w, proj_b, num_heads):
    assert int(num_heads) == NH
    _install_ntff_hook()
    in_maps = make_in_maps(x, gn_w, gn_b, qkv_w, proj_w, proj_b)
    if "nc" not in _CACHE:
        _CACHE["nc"] = build_nc(debug=DEBUG)
    r = run_bass_kernel_spmd(_CACHE["nc"], in_maps,
                             core_ids=list(range(NCORES)), trace=TRACE)
    _CACHE["last_result"] = r
    out = np.stack([np.asarray(r.results[b]["out"], dtype=np.float32)
                    for b in range(B)])
    return out.reshape(B, C, 32, 32)
